# revision 7
# baseline (speedup 1.0000x reference)
"""Echo State Network kernel for Trainium2 (8 NeuronCores, time-sharded).

Math (per reference):
    h_{t}   = tanh(W_in x_t + b_res + W_res h_{t-1}),  h in R^{2048}, T=1024
    y_t     = W_out h_t + b_out

Design — parallel-in-time with washout:
  The recurrence is weight-load bound on the PE (272 stationary 128x128
  tiles re-loaded every step), so per-step cost is nearly independent of
  the moving free dim (batch).  Data-parallel sharding therefore wastes
  cores.  Instead each core computes a T/8-step time chunk for the FULL
  batch (B=32), warmed up from h=0 over L=16 washout steps before its
  chunk.  The ESN contracts (spectral radius 0.9): the washout error at
  L=16 is ~1.3e-5 relative, far below the bf16 noise already present.
  Steps per core: 16 + 128 = 144 vs 1024 -> ~7x less PE work.

  - Fused contraction: W' = [W_res | W_in | b_res | 0] of shape
    [2048, 2176]; each step is 16 output chunks x 17 contraction chunks
    of [128x128] stationary tiles (bf16 -> compiler-automatic FWL),
    moving operand = state columns [128, 32].  Output lands
    reservoir-major; ACT applies tanh and casts to bf16.
  - Washout steps skip the states store; emit steps stream states to
    DRAM; a separate readout phase does y = W_out^T @ h at N=512.
  - t<0 washout entries (cores whose chunk starts near t=0) are
    zero-padded INCLUDING the bias-indicator row, so h stays exactly 0
    until the true t=0 -> core 0's chunk is bit-exact in methodology.
"""

from contextlib import ExitStack

import numpy as np
import ml_dtypes

import concourse.bass as bass
import concourse.tile as tile
from concourse import bacc, mybir
from concourse.bass import ds
from concourse.bass_utils import run_bass_kernel_spmd

BF16 = mybir.dt.bfloat16
F32 = mybir.dt.float32
AF = mybir.ActivationFunctionType

N_CORES = 8
B, T_FULL, N_IN, N_RES, N_OUT = 32, 1024, 64, 2048, 64
NCH = N_RES // 128         # 16 output chunks of 128
KCH = NCH + 1              # contraction chunks: 16 reservoir + 1 (x, bias)
KDIM = KCH * 128           # 2176 padded contraction size
L_WASH = 16                # washout steps (validated: rel err ~1.3e-5)
UNROLL = 2                 # steps per For_i iteration
RTB = 16                   # readout sub-block (steps) => N=512 matmuls

LAST_RESULTS = None        # BassKernelResults of the most recent run (for test.py)


def build_module(T=T_FULL, repeat=1):
    chunk = T // N_CORES
    assert chunk * N_CORES == T and chunk % UNROLL == 0
    nsteps = L_WASH + chunk

    nc = bacc.Bacc("TRN2")
    wt = nc.dram_tensor("wt", [KDIM, N_RES], BF16, kind="ExternalInput")
    xb = nc.dram_tensor("xb", [128, nsteps * B], BF16, kind="ExternalInput")
    wo = nc.dram_tensor("wo", [N_RES, N_OUT], BF16, kind="ExternalInput")
    bo = nc.dram_tensor("bo", [N_OUT, 1], F32, kind="ExternalInput")
    states = nc.dram_tensor("states", [chunk, 128, NCH * B], BF16)
    y = nc.dram_tensor("y", [N_OUT, chunk * B], F32, kind="ExternalOutput")

    with tile.TileContext(nc) as tc, ExitStack() as ctx:
        singles = ctx.enter_context(tc.tile_pool(name="singles", bufs=1))
        psum_pool = ctx.enter_context(
            tc.tile_pool(name="psum", bufs=2, space="PSUM")
        )

        w_sb = singles.tile([128, KCH * N_RES], BF16)
        nc.sync.dma_start(
            w_sb[:].rearrange("p (j n) -> p j n", n=N_RES),
            wt.rearrange("(j p) n -> p j n", p=128),
        )
        xb_sb = singles.tile([128, nsteps * B], BF16)
        nc.sync.dma_start(xb_sb[:], xb[:, :])
        wo_sb = singles.tile([128, NCH * N_OUT], BF16)
        nc.sync.dma_start(
            wo_sb[:].rearrange("p (k o) -> p k o", o=N_OUT),
            wo.rearrange("(k p) o -> p k o", p=128),
        )
        bo_sb = singles.tile([N_OUT, 1], F32)
        nc.sync.dma_start(bo_sb[:], bo[:, :])

        # Ping-pong state tiles, reservoir-major: H[p, B*j + b] = h[128j+p, b]
        H0 = singles.tile([128, NCH * B], BF16)
        H1 = singles.tile([128, NCH * B], BF16)
        nc.vector.memset(H0[:], 0.0)

        def w_tile(j, i):
            base = N_RES * j + 128 * i
            return w_sb[:, base : base + 128]

        def step(x_t_expr, Hsrc, Hdst, st_t_expr=None):
            ps = psum_pool.tile([128, NCH * B], F32, tag="ps")
            xcol = xb_sb[:, ds(x_t_expr * B, B)]
            for i in range(NCH):
                for j in range(KCH):
                    rhs = Hsrc[:, B * j : B * (j + 1)] if j < NCH else xcol
                    nc.tensor.matmul(
                        ps[:, B * i : B * (i + 1)],
                        w_tile(j, i),
                        rhs,
                        start=(j == 0),
                        stop=(j == KCH - 1),
                    )
            # Split tanh so the first half overlaps PE work on chunks 8-15
            # and the next step's early matmuls only wait on their half.
            half = NCH * B // 2
            nc.scalar.activation(Hdst[:, :half], ps[:, :half], AF.Tanh)
            nc.scalar.activation(Hdst[:, half:], ps[:, half:], AF.Tanh)
            if st_t_expr is not None:
                nc.sync.dma_start(
                    states[ds(st_t_expr, 1)].rearrange("o p f -> (o p) f"),
                    Hdst[:],
                )

        for _rep in range(repeat):
            if _rep > 0:
                nc.vector.memset(H0[:], 0.0)
            with tc.For_i(
                0, L_WASH, UNROLL, hint_engines=(mybir.EngineType.PE,)
            ) as iv:
                for s in range(UNROLL):
                    Hsrc, Hdst = (H0, H1) if s % 2 == 0 else (H1, H0)
                    step(iv + s, Hsrc, Hdst)
            with tc.For_i(
                0, chunk, UNROLL, hint_engines=(mybir.EngineType.PE,)
            ) as iv:
                for s in range(UNROLL):
                    Hsrc, Hdst = (H0, H1) if s % 2 == 0 else (H1, H0)
                    step(iv + s + L_WASH, Hsrc, Hdst, st_t_expr=iv + s)

        # Readout: y[o, (t, b)] = sum_n W_out[o, n] h_t[n, b] + b_out[o]
        st_pool = ctx.enter_context(tc.tile_pool(name="st", bufs=2))
        ysb_pool = ctx.enter_context(tc.tile_pool(name="ysb", bufs=2))
        ypsum_pool = ctx.enter_context(
            tc.tile_pool(name="ypsum", bufs=2, space="PSUM")
        )
        TBv = min(RTB, chunk)
        for tb in range(chunk // TBv):
            st = st_pool.tile([128, TBv * NCH * B], BF16, tag="st")
            nc.sync.dma_start(
                st[:].rearrange("p (t f) -> p t f", f=NCH * B),
                states[tb * TBv : (tb + 1) * TBv].rearrange("t p f -> p t f"),
            )
            st3 = st[:].rearrange("p (t f) -> p t f", f=NCH * B)
            yp = ypsum_pool.tile([N_OUT, TBv * B], F32, tag="yp")
            for k in range(NCH):
                nc.tensor.matmul(
                    yp[:],
                    wo_sb[:, N_OUT * k : N_OUT * (k + 1)],
                    st3[:, :, B * k : B * (k + 1)],
                    start=(k == 0),
                    stop=(k == NCH - 1),
                )
            ysb = ysb_pool.tile([N_OUT, TBv * B], F32, tag="ysb")
            nc.vector.tensor_scalar_add(ysb[:], yp[:], bo_sb[:, 0:1])
            nc.sync.dma_start(y[:, tb * TBv * B : (tb + 1) * TBv * B], ysb[:])

    nc.finalize()
    return nc


def prep_inputs(x, W_in, W_res, b_res, W_out, b_out, T=T_FULL):
    bf = ml_dtypes.bfloat16
    chunk = T // N_CORES
    nsteps = L_WASH + chunk
    Wp = np.concatenate(
        [
            W_res,
            W_in,
            b_res[:, None],
            np.zeros((N_RES, KDIM - N_RES - N_IN - 1), np.float32),
        ],
        axis=1,
    )
    wt = np.ascontiguousarray(Wp.T).astype(bf)            # [2176, 2048]
    wo = np.ascontiguousarray(W_out.T).astype(bf)         # [2048, 64]
    bo = np.ascontiguousarray(b_out[:, None]).astype(np.float32)
    in_maps = []
    for c in range(N_CORES):
        t0 = chunk * c
        lo = t0 - L_WASH
        xs = np.zeros((B, nsteps, N_IN), np.float32)      # [B, nsteps, N_IN]
        valid0 = max(0, -lo)                              # steps with t<0 stay 0
        xs[:, valid0:] = x[:, lo + valid0 : t0 + chunk]
        xbc = np.zeros((128, nsteps * B), bf)
        xbc[:N_IN] = (
            np.ascontiguousarray(xs.transpose(2, 1, 0).reshape(N_IN, nsteps * B))
            .astype(bf)
        )
        bias = np.ones((nsteps, B), np.float32)
        bias[:valid0] = 0.0                               # keep h == 0 before t=0
        xbc[N_IN] = bias.reshape(nsteps * B).astype(bf)
        in_maps.append({"wt": wt, "xb": xbc, "wo": wo, "bo": bo})
    return in_maps


def assemble_output(results, T=T_FULL):
    chunk = T // N_CORES
    y = np.empty((B, T, N_OUT), np.float32)
    for c in range(N_CORES):
        yc = results[c]["y"]                              # [64, chunk*B]
        y[:, chunk * c : chunk * (c + 1)] = (
            yc.reshape(N_OUT, chunk, B).transpose(2, 1, 0)
        )
    return y


def run(x, W_in, W_res, b_res, W_out, b_out, T=T_FULL, **run_kwargs):
    global LAST_RESULTS
    in_maps = prep_inputs(x, W_in, W_res, b_res, W_out, b_out, T=T)
    nc = build_module(T=T)
    res = run_bass_kernel_spmd(
        nc, in_maps, core_ids=list(range(N_CORES)), **run_kwargs
    )
    LAST_RESULTS = res
    return assemble_output(res.results, T=T)


def kernel(x, W_in, W_res, b_res, W_out, b_out):
    return run(
        np.asarray(x, np.float32),
        np.asarray(W_in, np.float32),
        np.asarray(W_res, np.float32),
        np.asarray(b_res, np.float32),
        np.asarray(W_out, np.float32),
        np.asarray(b_out, np.float32),
    )
